# revision 1
# baseline (speedup 1.0000x reference)
"""Trainium2 Bass kernel for CatFeaturesItemNet (EmbeddingBag sum, segment_reduce).

Strategy (data-parallel over items, table replicated — per sharding hint):
  * 8 cores, 8192 items each.
  * Host-side index prep (the "CSR side"): gather per-item feature ids
    (offsets/lengths/emb_bag_inputs are int32 index structures), sort each
    core's items by bag length (descending), build per-group gather streams.
  * Device does all f32 payload movement + reduction:
      - weight table viewed as [25000, 512] f32 (2KB blocks of 4 rows) so
        block ids fit dma_gather's int16 index limit (id>>2 < 25000).
      - per group of 128 items: one dma_gather pulls L lanes/item
        (column-major: lane t of item p lands at dest[p, t, :512]).
      - DVE: in-place multiply by uploaded phase-mask (selects the right
        128-f32 row of each 2KB block, zeroes padding lanes), then
        tensor_reduce over (lane, phase) -> [128 items, 128] f32.
      - store per group; host unpermutes rows back to original item order.
  * Lane budget per group is a static schedule derived from binomial bounds
    on sorted uniform{1..16} lengths — program shape is input-independent;
    only tensor contents depend on the inputs.
"""

import numpy as np
from contextlib import ExitStack

N_CORES = 8
BATCH = 65536
BL = BATCH // N_CORES          # items per core
L_MAX = 16
D = 128
V = 100000                     # weight rows
NBLK = V // 4                  # 2KB blocks (4 rows each)
ELEM = 512                     # f32 per gathered block (2KB)
GROUPS = BL // 128             # 64 groups of 128 items per core
SAFETY_SIGMA = 10.0


def _static_lane_schedule(n_items=BL, groups=GROUPS):
    """L_hat[g]: static upper bound on the max bag length within group g of
    128 items after sorting lengths (uniform{1..16}) in descending order.
    Group g's max length exceeds L only if count(len >= L+1) > 128*g;
    count(len >= k) ~ Binomial(n, (17-k)/16)."""
    sched = []
    for g in range(groups):
        lhat = L_MAX
        for L in range(1, L_MAX + 1):
            p = (L_MAX - L) / 16.0  # P(len >= L+1) for len ~ uniform{1..16}
            mean = n_items * p
            sigma = np.sqrt(n_items * p * (1 - p))
            if mean + SAFETY_SIGMA * sigma <= g * 128:
                lhat = L
                break
        sched.append(lhat)
    return sched


L_SCHED = _static_lane_schedule()


def _build_bass():
    import concourse.bass as bass
    import concourse.bacc as bacc
    import concourse.tile as tile
    from concourse import mybir
    from concourse.library_config import mlp

    idx_cols = sum(128 * L // 16 for L in L_SCHED)        # int16 cols
    mask_cols = sum(L * 4 for L in L_SCHED)               # f32 cols

    nc = bacc.Bacc("TRN2", target_bir_lowering=False, debug=False,
                   num_devices=N_CORES)
    weight = nc.declare_dram_parameter("weight", [V, D], mybir.dt.float32,
                                       isOutput=False)
    idx_in = nc.declare_dram_parameter("idx", [128, idx_cols], mybir.dt.int16,
                                       isOutput=False)
    mask_in = nc.declare_dram_parameter("mask", [128, mask_cols],
                                        mybir.dt.float32, isOutput=False)
    out = nc.declare_dram_parameter("out", [BL, D], mybir.dt.float32,
                                    isOutput=True)

    wblk = weight.rearrange("(a b) d -> a (b d)", b=4)    # [25000, 512]
    out_g = out.rearrange("(g p) d -> g p d", p=128)      # [64, 128, 128]

    with tile.TileContext(nc) as tc:
        with ExitStack() as ctx:
            cons = ctx.enter_context(tc.tile_pool(name="cons", bufs=1))
            gp = ctx.enter_context(tc.tile_pool(name="g", bufs=4))
            op = ctx.enter_context(tc.tile_pool(name="o", bufs=3))

            nc.gpsimd.load_library(mlp)
            idx_t = cons.tile([128, idx_cols], mybir.dt.int16)
            nc.gpsimd.dma_start(out=idx_t[:], in_=idx_in[:, :])
            mask_t = cons.tile([128, mask_cols], mybir.dt.float32)
            nc.sync.dma_start(out=mask_t[:], in_=mask_in[:, :])

            ic = 0   # running idx col offset
            mc = 0   # running mask col offset
            for g, L in enumerate(L_SCHED):
                ni = 128 * L
                gt = gp.tile([128, L_MAX * ELEM], mybir.dt.float32, tag="g")
                gv = gt[:, :L * ELEM]
                nc.gpsimd.dma_gather(
                    out_ap=gv.rearrange("p (c e) -> p c e", e=ELEM),
                    in_ap=wblk[:, :],
                    idxs_ap=idx_t[:, ic:ic + ni // 16],
                    num_idxs=ni,
                    num_idxs_reg=ni,
                    elem_size=ELEM,
                    single_packet=False,
                )
                # select phase row + zero pad lanes:  G *= M (broadcast over d)
                g4 = gv.rearrange("p (t q d) -> p t q d", q=4, d=D)
                m4 = mask_t[:, mc:mc + L * 4].rearrange(
                    "p (t q) -> p t q", q=4).to_broadcast([128, L, 4, D])
                nc.vector.tensor_tensor(out=g4, in0=g4, in1=m4,
                                        op=mybir.AluOpType.mult)
                # sum over (t, q): innermost two dims of [p][d][t][q]
                o_t = op.tile([128, D], mybir.dt.float32, tag="o")
                rin = gv.rearrange("p (t q d) -> p d t q", q=4, d=D)
                nc.vector.tensor_reduce(out=o_t[:], in_=rin,
                                        axis=mybir.AxisListType.XY,
                                        op=mybir.AluOpType.add)
                nc.sync.dma_start(out=out_g[g], in_=o_t[:])
                ic += ni // 16
                mc += L * 4
    nc.compile()
    return nc, idx_cols, mask_cols


def _host_prep(core_items, emb_bag_inputs, offsets, input_lengths,
               idx_cols, mask_cols):
    """Build per-core idx/mask tensors + the inverse permutation."""
    it = core_items.astype(np.int64)
    off = offsets[it].astype(np.int64)
    ln = input_lengths[it].astype(np.int64)
    ids = emb_bag_inputs[off[:, None] + np.arange(L_MAX)[None, :]].astype(np.int64)

    order = np.argsort(-ln, kind="stable")      # items sorted by len desc
    ln_s = ln[order]
    ids_s = ids[order]

    idx_arr = np.zeros((128, idx_cols), dtype=np.int16)
    mask_arr = np.zeros((128, mask_cols), dtype=np.float32)
    ic = 0
    mc = 0
    for g, L in enumerate(L_SCHED):
        sl = slice(g * 128, (g + 1) * 128)
        ln_g = ln_s[sl]                          # [128]
        if ln_g.max(initial=0) > L:
            raise RuntimeError(
                f"static lane schedule violated in group {g}: "
                f"max len {ln_g.max()} > {L}")
        ids_g = ids_s[sl]                        # [128, 16]
        lanes = np.minimum(np.arange(L)[None, :], ln_g[:, None] - 1)  # pad->dup lane0.. actually dup of clamped lane
        lane_ids = np.take_along_axis(ids_g, lanes, axis=1)  # [128, L]
        blk = (lane_ids >> 2).astype(np.int16)               # [128, L]
        ph = (lane_ids & 3).astype(np.int64)                 # [128, L]
        valid = (np.arange(L)[None, :] < ln_g[:, None])      # [128, L]

        # column-major stream: s = t*128 + p
        stream = blk.T.reshape(-1)                           # [128*L]
        ni = 128 * L
        wrapped = stream.reshape(ni // 16, 16).T             # [16, ni/16]
        idx_arr[:, ic:ic + ni // 16] = np.tile(wrapped, (8, 1))

        m = np.zeros((128, L, 4), dtype=np.float32)
        np.put_along_axis(m, ph[:, :, None], 1.0, axis=2)
        m *= valid[:, :, None]
        mask_arr[:, mc:mc + L * 4] = m.reshape(128, L * 4)
        ic += ni // 16
        mc += L * 4

    inv = np.empty(BL, dtype=np.int64)
    inv[order] = np.arange(BL)                  # original j -> sorted row
    return idx_arr, mask_arr, inv


_CACHE = {}


def kernel(items, emb_bag_inputs, offsets, input_lengths, weight):
    from concourse.bass_utils import run_bass_kernel_spmd

    if "nc" not in _CACHE:
        _CACHE["nc"], _CACHE["icols"], _CACHE["mcols"] = _build_bass()
    nc = _CACHE["nc"]
    icols, mcols = _CACHE["icols"], _CACHE["mcols"]

    weight_f32 = np.ascontiguousarray(weight, dtype=np.float32)
    in_maps = []
    invs = []
    for c in range(N_CORES):
        idx_arr, mask_arr, inv = _host_prep(
            np.asarray(items[c * BL:(c + 1) * BL]),
            np.asarray(emb_bag_inputs), np.asarray(offsets),
            np.asarray(input_lengths), icols, mcols)
        in_maps.append({"weight": weight_f32, "idx": idx_arr,
                        "mask": mask_arr})
        invs.append(inv)

    res = run_bass_kernel_spmd(nc, in_maps, list(range(N_CORES)))
    outs = []
    for c in range(N_CORES):
        dev = res.results[c]["out"]            # [BL, D] in sorted order
        outs.append(dev[invs[c]])
    return np.concatenate(outs, axis=0).astype(np.float32)



# revision 14
# speedup vs baseline: 9.2717x; 9.2717x over previous
"""Trainium2 Bass kernel for CatFeaturesItemNet (EmbeddingBag sum, segment_reduce).

Strategy (table-quarter x batch-half grid over 8 cores):
  * Core c handles batch half h = c//4 (32768 items) and vocab quarter
    q = c%4 (25000 rows, padded to 25088 with a zero row used for lane
    padding -> no masks needed anywhere).
  * Host prep per core: gather each item's feature ids, keep the ones in
    quarter q (k_i of them), left-pack, sort items by k_i descending, pad
    each group of 128 items to a static per-group lane budget L_SCHED[g]
    (binomial order-statistic bound; program shape is input-independent).
  * Device: the quarter table is stored as bf16 (either SBUF-resident or
    read directly from HBM); per super-group one dma_gather with
    transpose=True pulls 256B rows => dest[d, slot] = W[token[slot], d]
    with slots item-major, so a contiguous innermost tensor_reduce gives
    the per-item bag sums [d, 128 items] in f32. 4 SWDGE queues round-robin
    (measured ~2-3 ns/descriptor vs ~8 single-queue).
  * Output [128 d, 32768 sorted items] f32 per core; host adds the four
    quarter partials per half and unsorts.
"""

import numpy as np
import ml_dtypes
from contextlib import ExitStack
from math import comb

N_CORES = 8
BATCH = 65536
HALF = BATCH // 2              # items per core (two cores share each half)
L_MAX = 16
D = 128
V = 100000
QROWS = V // 4                 # 25000 rows per vocab quarter
QPAD = 25088                   # padded to rank multiple of 128 (196 ranks)
ZTOK = QROWS                   # first padded row: all-zero, used for padding
GROUPS = HALF // 128           # 256 groups of 128 items
SAFETY_SIGMA = 10.0
MAX_SUPER_SLOTS = 4096         # per-gather-call slot cap (num_idxs)
MAX_SUPER_GROUPS = 8           # per-call group cap (bounds the psum tile)
import os as _os
NQUEUES = int(_os.environ.get("KNQ", "4"))


def _lane_schedule():
    """L_hat[g]: static bound on max per-quarter lane count within group g of
    128 items after sorting counts descending. count k_i ~ Binomial(len_i, 1/4)
    with len_i ~ Uniform{1..16}; count(k >= K) ~ Binomial(HALF, p_K)."""
    def p_ge(K):
        tot = 0.0
        for ln in range(1, L_MAX + 1):
            tot += sum(comb(ln, j) * 0.25 ** j * 0.75 ** (ln - j)
                       for j in range(K, ln + 1)) / 16.0
        return tot

    sched = []
    for g in range(GROUPS):
        lhat = L_MAX
        for K in range(0, L_MAX + 1):
            p = p_ge(K + 1)
            mean = HALF * p
            sig = np.sqrt(HALF * p * (1 - p))
            if mean + SAFETY_SIGMA * sig <= g * 128:
                lhat = K
                break
        sched.append(lhat)
    return sched


L_SCHED = _lane_schedule()


def _super_plan(uniform=False):
    """Pack consecutive non-empty groups into gather calls obeying the slot
    and group caps. uniform=True additionally requires equal L within a call
    (enables contiguous tree-reduction). Returns list of (g0, g1, nidx)."""
    plan = []
    g = 0
    while g < GROUPS and L_SCHED[g] > 0:
        g0 = g
        slots = 0
        while (g < GROUPS and L_SCHED[g] > 0 and g - g0 < MAX_SUPER_GROUPS
               and slots + 128 * L_SCHED[g] <= MAX_SUPER_SLOTS
               and (not uniform or L_SCHED[g] == L_SCHED[g0])):
            slots += 128 * L_SCHED[g]
            g += 1
        plan.append((g0, g, slots))
    return plan


SUPERS = _super_plan()
SLOTS = sum(s for _, _, s in SUPERS)
ICOLS = SLOTS // 16
USUPERS = _super_plan(uniform=True)
USLOTS = sum(s for _, _, s in USUPERS)
UICOLS = USLOTS // 16


def _build_bass(variant="hbm", replicas=1):
    import concourse.bass as bass
    import concourse.bacc as bacc
    import concourse.tile as tile
    from concourse import mybir
    from concourse.library_config import mlp

    nc = bacc.Bacc("TRN2", target_bir_lowering=False, debug=False,
                   num_devices=N_CORES, num_swdge_queues=NQUEUES)
    if variant == "sbuf":
        wsb = nc.declare_dram_parameter("wsb", [128, QPAD],
                                        mybir.dt.bfloat16, isOutput=False)
    elif variant == "hbm":
        wq = nc.declare_dram_parameter("wq", [QPAD, D], mybir.dt.bfloat16,
                                       isOutput=False)
    else:  # q512: f32 rows, non-transpose gather, tree reduce
        wq = nc.declare_dram_parameter("wq", [QPAD, D], mybir.dt.float32,
                                       isOutput=False)
    supers = USUPERS if variant == "q512" else SUPERS
    icols = UICOLS if variant == "q512" else ICOLS
    idx_in = nc.declare_dram_parameter("idx", [128, icols], mybir.dt.int16,
                                       isOutput=False)
    if variant == "q512":
        out = nc.declare_dram_parameter("out", [HALF, D], mybir.dt.float32,
                                        isOutput=True)
        out_v = out.rearrange("(G p) d -> p G d", p=128)   # [128, GROUPS, D]
    else:
        out = nc.declare_dram_parameter("out", [128, HALF], mybir.dt.float32,
                                        isOutput=True)

    with tile.TileContext(nc) as tc:
        with ExitStack() as ctx:
            cons = ctx.enter_context(tc.tile_pool(name="cons", bufs=1))
            gp = ctx.enter_context(tc.tile_pool(name="g", bufs=6))
            op = ctx.enter_context(tc.tile_pool(name="o", bufs=4))

            nc.gpsimd.load_library(mlp)
            idx_t = cons.tile([128, icols], mybir.dt.int16)
            nc.sync.dma_start(out=idx_t[:], in_=idx_in[:, :])

            def body():
                if variant == "sbuf":
                    wtab = cons.tile([128, QPAD], mybir.dt.bfloat16,
                                     tag="wtab")
                    nc.sync.dma_start(out=wtab[:], in_=wsb[:, :])
                ic = 0
                for si, (g0, g1, nidx) in enumerate(supers):
                    qn = si % NQUEUES
                    if variant == "q512":
                        L = L_SCHED[g0]
                        ng = g1 - g0
                        gt = gp.tile([128, MAX_SUPER_SLOTS], mybir.dt.float32,
                                     tag="g")
                        gv = gt[:, :nidx]
                        nc.gpsimd.dma_gather(
                            out_ap=gv.rearrange("p (c e) -> p c e", e=D),
                            in_ap=wq[:, :],
                            idxs_ap=idx_t[:, ic:ic + nidx // 16],
                            num_idxs=nidx, num_idxs_reg=nidx,
                            elem_size=D, single_packet=False, queue_num=qn)
                        # contiguous tree-reduce over the lane axis t
                        v4 = gv.rearrange("p (g t e) -> p g t e", t=L, e=D)
                        cur = L
                        while cur > 1:
                            lo = (cur + 1) // 2
                            h = cur - lo
                            nc.vector.tensor_tensor(
                                out=v4[:, :, 0:h, :], in0=v4[:, :, 0:h, :],
                                in1=v4[:, :, lo:lo + h, :],
                                op=mybir.AluOpType.add)
                            cur = lo
                        ot = op.tile([128, 128 * MAX_SUPER_GROUPS],
                                     mybir.dt.float32, tag="o")
                        nc.vector.tensor_copy(out=ot[:, :ng * D].rearrange(
                            "p (g e) -> p g e", e=D), in_=v4[:, :, 0, :])
                        nc.sync.dma_start(out=out_v[:, g0:g1, :],
                                          in_=ot[:, :ng * D].rearrange(
                                              "p (g e) -> p g e", e=D))
                        ic += nidx // 16
                        continue
                    gt = gp.tile([128, MAX_SUPER_SLOTS], mybir.dt.bfloat16,
                                 tag="g")
                    gv = gt[:, :nidx]
                    kw = dict(
                        out_ap=gv.rearrange("p (c n) -> p c n", c=1),
                        idxs_ap=idx_t[:, ic:ic + nidx // 16],
                        num_idxs=nidx, num_idxs_reg=nidx,
                        elem_size=128, transpose=True, single_packet=False,
                        queue_num=qn)
                    if variant == "sbuf":
                        nc.gpsimd.dma_gather(
                            in_ap=wtab[:, :], sbuf_tokens_per_rank=128,
                            sbuf_free_dim_per_rank=256, **kw)
                    else:
                        nc.gpsimd.dma_gather(in_ap=wq[:, :], **kw)
                    ot = op.tile([128, 128 * MAX_SUPER_GROUPS],
                                 mybir.dt.float32, tag="o")
                    off = 0
                    for g in range(g0, g1):
                        L = L_SCHED[g]
                        view = gv[:, off:off + 128 * L].rearrange(
                            "d (p t) -> d p t", t=L)
                        nc.vector.tensor_reduce(
                            out=ot[:, (g - g0) * 128:(g - g0 + 1) * 128],
                            in_=view, axis=mybir.AxisListType.X,
                            op=mybir.AluOpType.add)
                        off += 128 * L
                    nc.sync.dma_start(
                        out=out[:, g0 * 128:g1 * 128],
                        in_=ot[:, :(g1 - g0) * 128])
                    ic += nidx // 16

            if replicas == 1:
                body()
            else:
                with tc.For_i(0, replicas):
                    body()
    nc.compile()
    return nc


def _host_prep(items_half, emb_bag_inputs, offsets, input_lengths, quarter,
               variant="hbm"):
    """Per-core index prep. Returns (idx_arr [128, icols] int16, order)."""
    it = items_half.astype(np.int64)
    off = offsets[it].astype(np.int64)
    ln = input_lengths[it].astype(np.int64)
    ids = emb_bag_inputs[off[:, None] + np.arange(L_MAX)[None, :]].astype(
        np.int64)                                          # [N, 16]
    valid = np.arange(L_MAX)[None, :] < ln[:, None]
    inq = valid & (ids // QROWS == quarter)
    local = np.where(inq, ids - QROWS * quarter, 30000)    # pad sorts last
    packed = np.sort(local, axis=1)                        # [N, 16]
    k = inq.sum(axis=1)

    order = np.argsort(-k, kind="stable")
    packed_s = packed[order]
    k_s = k[order]

    supers = USUPERS if variant == "q512" else SUPERS
    icols = UICOLS if variant == "q512" else ICOLS
    idx_arr = np.zeros((128, icols), dtype=np.int16)
    ic = 0
    for g0, g1, nidx in supers:
        for g in range(g0, g1):
            if k_s[g * 128] > L_SCHED[g]:
                raise RuntimeError(
                    f"lane schedule violated: group {g} max k {k_s[g*128]} "
                    f"> {L_SCHED[g]}")
        if variant == "q512":
            L = L_SCHED[g0]
            ng = g1 - g0
            slabs = packed_s[g0 * 128:g1 * 128, :L]
            slabs = np.where(slabs >= QROWS, ZTOK, slabs)
            # stream s = c*128 + p, c = gi*L + t (column-major slots)
            stream = slabs.reshape(ng, 128, L).transpose(0, 2, 1).reshape(-1)
        else:
            streams = []
            for g in range(g0, g1):
                L = L_SCHED[g]
                slab = packed_s[g * 128:(g + 1) * 128, :L]
                slab = np.where(slab >= QROWS, ZTOK, slab)
                streams.append(slab.reshape(-1))           # item-major
            stream = np.concatenate(streams)
        wrapped = stream.reshape(nidx // 16, 16).T         # [16, nidx/16]
        idx_arr[:, ic:ic + nidx // 16] = np.tile(wrapped, (8, 1))
        ic += nidx // 16
    return idx_arr.astype(np.int16), order


def _prep_tables(weight, variant):
    """Per-quarter tables in the layout the device expects."""
    dt = np.float32 if variant == "q512" else ml_dtypes.bfloat16
    w16 = weight.astype(dt)
    tabs = []
    for q in range(4):
        wq = np.zeros((QPAD, D), dtype=dt)
        wq[:QROWS] = w16[q * QROWS:(q + 1) * QROWS]
        if variant == "sbuf":
            # wtab[p, r*128 + d] = wq[r*128 + p, d]
            tabs.append(np.ascontiguousarray(
                wq.reshape(QPAD // 128, 128, D).transpose(1, 0, 2)
                .reshape(128, QPAD)))
        else:
            tabs.append(wq)
    return tabs


_CACHE = {}
VARIANT = "q512"


def kernel(items, emb_bag_inputs, offsets, input_lengths, weight):
    from concourse.bass_utils import run_bass_kernel_spmd

    variant = VARIANT
    key = ("nc", variant)
    if key not in _CACHE:
        _CACHE[key] = _build_bass(variant)
    nc = _CACHE[key]

    items = np.asarray(items)
    emb_bag_inputs = np.asarray(emb_bag_inputs)
    offsets = np.asarray(offsets)
    input_lengths = np.asarray(input_lengths)
    weight = np.asarray(weight, dtype=np.float32)

    tabs = _prep_tables(weight, variant)
    in_maps = []
    orders = []
    for c in range(N_CORES):
        h, q = c // 4, c % 4
        idx_arr, order = _host_prep(items[h * HALF:(h + 1) * HALF],
                                    emb_bag_inputs, offsets, input_lengths, q,
                                    variant)
        m = {"idx": idx_arr}
        if variant == "sbuf":
            m["wsb"] = tabs[q]
        else:
            m["wq"] = tabs[q]
        in_maps.append(m)
        orders.append(order)

    res = run_bass_kernel_spmd(nc, in_maps, list(range(N_CORES)))

    full = np.empty((BATCH, D), dtype=np.float32)
    for h in range(2):
        acc = np.zeros((HALF, D), dtype=np.float32)
        for q in range(4):
            c = h * 4 + q
            part = res.results[c]["out"]
            acc[orders[c]] += (part if variant == "q512" else part.T)
        full[h * HALF:(h + 1) * HALF] = acc
    return full


# revision 20
# speedup vs baseline: 16.5176x; 1.7815x over previous
"""Trainium2 Bass kernel for CatFeaturesItemNet (EmbeddingBag sum, segment_reduce).

Strategy (table-quarter x batch-half grid over 8 cores):
  * Core c handles batch half h = c//4 (32768 items) and vocab quarter
    q = c%4 (25000 rows, padded to 25088 with a zero row used for lane
    padding -> no masks needed anywhere).
  * Host prep per core: gather each item's feature ids, keep the ones in
    quarter q (k_i of them), left-pack, sort items by k_i descending, pad
    each group of 128 items to a static per-group lane budget L_SCHED[g]
    (binomial order-statistic bound; program shape is input-independent).
  * Device: the quarter table is stored as bf16 (either SBUF-resident or
    read directly from HBM); per super-group one dma_gather with
    transpose=True pulls 256B rows => dest[d, slot] = W[token[slot], d]
    with slots item-major, so a contiguous innermost tensor_reduce gives
    the per-item bag sums [d, 128 items] in f32. 4 SWDGE queues round-robin
    (measured ~2-3 ns/descriptor vs ~8 single-queue).
  * Output [128 d, 32768 sorted items] f32 per core; host adds the four
    quarter partials per half and unsorts.
"""

import numpy as np
import ml_dtypes
from contextlib import ExitStack
from math import comb

N_CORES = 8
BATCH = 65536
HALF = BATCH // 2              # items per core (two cores share each half)
L_MAX = 16
D = 128
V = 100000
QROWS = V // 4                 # 25000 rows per vocab quarter
QPAD = 25088                   # padded to rank multiple of 128 (196 ranks)
ZTOK = QROWS                   # first padded row: all-zero, used for padding
GROUPS = HALF // 128           # 256 groups of 128 items
SAFETY_SIGMA = 10.0
MAX_SUPER_SLOTS = 4096         # per-gather-call slot cap (num_idxs)
MAX_SUPER_GROUPS = 8           # per-call group cap (bounds the psum tile)
import os as _os
NQUEUES = int(_os.environ.get("KNQ", "4"))


def _lane_schedule():
    """L_hat[g]: static bound on max per-quarter lane count within group g of
    128 items after sorting counts descending. count k_i ~ Binomial(len_i, 1/4)
    with len_i ~ Uniform{1..16}; count(k >= K) ~ Binomial(HALF, p_K)."""
    def p_ge(K):
        tot = 0.0
        for ln in range(1, L_MAX + 1):
            tot += sum(comb(ln, j) * 0.25 ** j * 0.75 ** (ln - j)
                       for j in range(K, ln + 1)) / 16.0
        return tot

    sched = []
    for g in range(GROUPS):
        lhat = L_MAX
        for K in range(0, L_MAX + 1):
            p = p_ge(K + 1)
            mean = HALF * p
            sig = np.sqrt(HALF * p * (1 - p))
            if mean + SAFETY_SIGMA * sig <= g * 128:
                lhat = K
                break
        sched.append(lhat)
    return sched


L_SCHED = _lane_schedule()


def _super_plan(uniform=False):
    """Pack consecutive non-empty groups into gather calls obeying the slot
    and group caps. uniform=True additionally requires equal L within a call
    (enables contiguous tree-reduction). Returns list of (g0, g1, nidx)."""
    plan = []
    g = 0
    while g < GROUPS and L_SCHED[g] > 0:
        g0 = g
        slots = 0
        while (g < GROUPS and L_SCHED[g] > 0 and g - g0 < MAX_SUPER_GROUPS
               and slots + 128 * L_SCHED[g] <= MAX_SUPER_SLOTS
               and (not uniform or L_SCHED[g] == L_SCHED[g0])):
            slots += 128 * L_SCHED[g]
            g += 1
        plan.append((g0, g, slots))
    return plan


SUPERS = _super_plan()
SLOTS = sum(s for _, _, s in SUPERS)
ICOLS = SLOTS // 16
USUPERS = _super_plan(uniform=True)
USLOTS = sum(s for _, _, s in USUPERS)
UICOLS = USLOTS // 16


def _build_bass(variant="hbm", replicas=1):
    import concourse.bass as bass
    import concourse.bacc as bacc
    import concourse.tile as tile
    from concourse import mybir
    from concourse.library_config import mlp

    nc = bacc.Bacc("TRN2", target_bir_lowering=False, debug=False,
                   num_devices=N_CORES, num_swdge_queues=NQUEUES)
    if variant == "sbuf":
        wsb = nc.declare_dram_parameter("wsb", [128, QPAD],
                                        mybir.dt.bfloat16, isOutput=False)
    elif variant == "hbm":
        wq = nc.declare_dram_parameter("wq", [QPAD, D], mybir.dt.bfloat16,
                                       isOutput=False)
    else:  # q512/q256: f32/bf16 rows, non-transpose gather, tree reduce
        gdt = mybir.dt.float32 if variant == "q512" else mybir.dt.bfloat16
        wq = nc.declare_dram_parameter("wq", [QPAD, D], gdt, isOutput=False)
    tree = variant in ("q512", "q256")
    supers = USUPERS if tree else SUPERS
    icols = UICOLS if tree else ICOLS
    idx_in = nc.declare_dram_parameter("idx", [128, icols], mybir.dt.int16,
                                       isOutput=False)
    if tree:
        out = nc.declare_dram_parameter("out", [HALF, D], mybir.dt.float32,
                                        isOutput=True)
        out_v = out.rearrange("(G p) d -> p G d", p=128)   # [128, GROUPS, D]
    else:
        out = nc.declare_dram_parameter("out", [128, HALF], mybir.dt.float32,
                                        isOutput=True)

    with tile.TileContext(nc) as tc:
        with ExitStack() as ctx:
            cons = ctx.enter_context(tc.tile_pool(name="cons", bufs=1))
            gp = ctx.enter_context(
                tc.tile_pool(name="g", bufs=10 if variant == "q256" else 6))
            op = ctx.enter_context(tc.tile_pool(name="o", bufs=4))

            nc.gpsimd.load_library(mlp)
            idx_t = cons.tile([128, icols], mybir.dt.int16)
            nc.sync.dma_start(out=idx_t[:], in_=idx_in[:, :])

            def body():
                if variant == "sbuf":
                    wtab = cons.tile([128, QPAD], mybir.dt.bfloat16,
                                     tag="wtab")
                    nc.sync.dma_start(out=wtab[:], in_=wsb[:, :])
                ic = 0
                for si, (g0, g1, nidx) in enumerate(supers):
                    qn = si % NQUEUES
                    if tree:
                        L = L_SCHED[g0]
                        ng = g1 - g0
                        gt = gp.tile([128, MAX_SUPER_SLOTS], gdt, tag="g")
                        gv = gt[:, :nidx]
                        nc.gpsimd.dma_gather(
                            out_ap=gv.rearrange("p (c e) -> p c e", e=D),
                            in_ap=wq[:, :],
                            idxs_ap=idx_t[:, ic:ic + nidx // 16],
                            num_idxs=nidx, num_idxs_reg=nidx,
                            elem_size=D, single_packet=False, queue_num=qn)
                        # contiguous tree-reduce over the lane axis t
                        v4 = gv.rearrange("p (g t e) -> p g t e", t=L, e=D)
                        cur = L
                        while cur > 1:
                            lo = (cur + 1) // 2
                            h = cur - lo
                            nc.vector.tensor_tensor(
                                out=v4[:, :, 0:h, :], in0=v4[:, :, 0:h, :],
                                in1=v4[:, :, lo:lo + h, :],
                                op=mybir.AluOpType.add)
                            cur = lo
                        ot = op.tile([128, 128 * MAX_SUPER_GROUPS],
                                     mybir.dt.float32, tag="o")
                        nc.vector.tensor_copy(out=ot[:, :ng * D].rearrange(
                            "p (g e) -> p g e", e=D), in_=v4[:, :, 0, :])
                        nc.sync.dma_start(out=out_v[:, g0:g1, :],
                                          in_=ot[:, :ng * D].rearrange(
                                              "p (g e) -> p g e", e=D))
                        ic += nidx // 16
                        continue
                    gt = gp.tile([128, MAX_SUPER_SLOTS], mybir.dt.bfloat16,
                                 tag="g")
                    gv = gt[:, :nidx]
                    kw = dict(
                        out_ap=gv.rearrange("p (c n) -> p c n", c=1),
                        idxs_ap=idx_t[:, ic:ic + nidx // 16],
                        num_idxs=nidx, num_idxs_reg=nidx,
                        elem_size=128, transpose=True, single_packet=False,
                        queue_num=qn)
                    if variant == "sbuf":
                        nc.gpsimd.dma_gather(
                            in_ap=wtab[:, :], sbuf_tokens_per_rank=128,
                            sbuf_free_dim_per_rank=256, **kw)
                    else:
                        nc.gpsimd.dma_gather(in_ap=wq[:, :], **kw)
                    ot = op.tile([128, 128 * MAX_SUPER_GROUPS],
                                 mybir.dt.float32, tag="o")
                    off = 0
                    for g in range(g0, g1):
                        L = L_SCHED[g]
                        view = gv[:, off:off + 128 * L].rearrange(
                            "d (p t) -> d p t", t=L)
                        nc.vector.tensor_reduce(
                            out=ot[:, (g - g0) * 128:(g - g0 + 1) * 128],
                            in_=view, axis=mybir.AxisListType.X,
                            op=mybir.AluOpType.add)
                        off += 128 * L
                    nc.sync.dma_start(
                        out=out[:, g0 * 128:g1 * 128],
                        in_=ot[:, :(g1 - g0) * 128])
                    ic += nidx // 16

            if replicas == 1:
                body()
            else:
                with tc.For_i(0, replicas):
                    body()
    nc.compile()
    return nc


def _host_prep(items_half, emb_bag_inputs, offsets, input_lengths, quarter,
               variant="hbm"):
    """Per-core index prep. Returns (idx_arr [128, icols] int16, order)."""
    it = items_half.astype(np.int64)
    off = offsets[it].astype(np.int64)
    ln = input_lengths[it].astype(np.int64)
    ids = emb_bag_inputs[off[:, None] + np.arange(L_MAX)[None, :]].astype(
        np.int64)                                          # [N, 16]
    valid = np.arange(L_MAX)[None, :] < ln[:, None]
    inq = valid & (ids // QROWS == quarter)
    local = np.where(inq, ids - QROWS * quarter, 30000)    # pad sorts last
    packed = np.sort(local, axis=1)                        # [N, 16]
    k = inq.sum(axis=1)

    order = np.argsort(-k, kind="stable")
    packed_s = packed[order]
    k_s = k[order]

    tree = variant in ("q512", "q256")
    supers = USUPERS if tree else SUPERS
    icols = UICOLS if tree else ICOLS
    idx_arr = np.zeros((128, icols), dtype=np.int16)
    ic = 0
    for g0, g1, nidx in supers:
        for g in range(g0, g1):
            if k_s[g * 128] > L_SCHED[g]:
                raise RuntimeError(
                    f"lane schedule violated: group {g} max k {k_s[g*128]} "
                    f"> {L_SCHED[g]}")
        if tree:
            L = L_SCHED[g0]
            ng = g1 - g0
            slabs = packed_s[g0 * 128:g1 * 128, :L]
            slabs = np.where(slabs >= QROWS, ZTOK, slabs)
            # stream s = c*128 + p, c = gi*L + t (column-major slots)
            stream = slabs.reshape(ng, 128, L).transpose(0, 2, 1).reshape(-1)
        else:
            streams = []
            for g in range(g0, g1):
                L = L_SCHED[g]
                slab = packed_s[g * 128:(g + 1) * 128, :L]
                slab = np.where(slab >= QROWS, ZTOK, slab)
                streams.append(slab.reshape(-1))           # item-major
            stream = np.concatenate(streams)
        wrapped = stream.reshape(nidx // 16, 16).T         # [16, nidx/16]
        idx_arr[:, ic:ic + nidx // 16] = np.tile(wrapped, (8, 1))
        ic += nidx // 16
    return idx_arr.astype(np.int16), order


def _prep_tables(weight, variant):
    """Per-quarter tables in the layout the device expects."""
    dt = np.float32 if variant == "q512" else ml_dtypes.bfloat16  # q256: bf16
    w16 = weight.astype(dt)
    tabs = []
    for q in range(4):
        wq = np.zeros((QPAD, D), dtype=dt)
        wq[:QROWS] = w16[q * QROWS:(q + 1) * QROWS]
        if variant == "sbuf":
            # wtab[p, r*128 + d] = wq[r*128 + p, d]
            tabs.append(np.ascontiguousarray(
                wq.reshape(QPAD // 128, 128, D).transpose(1, 0, 2)
                .reshape(128, QPAD)))
        else:
            tabs.append(wq)
    return tabs


_CACHE = {}
VARIANT = "q512"


def kernel(items, emb_bag_inputs, offsets, input_lengths, weight):
    from concourse.bass_utils import run_bass_kernel_spmd

    variant = VARIANT
    key = ("nc", variant)
    if key not in _CACHE:
        _CACHE[key] = _build_bass(variant)
    nc = _CACHE[key]

    items = np.asarray(items)
    emb_bag_inputs = np.asarray(emb_bag_inputs)
    offsets = np.asarray(offsets)
    input_lengths = np.asarray(input_lengths)
    weight = np.asarray(weight, dtype=np.float32)

    tabs = _prep_tables(weight, variant)
    in_maps = []
    orders = []
    for c in range(N_CORES):
        h, q = c // 4, c % 4
        idx_arr, order = _host_prep(items[h * HALF:(h + 1) * HALF],
                                    emb_bag_inputs, offsets, input_lengths, q,
                                    variant)
        m = {"idx": idx_arr}
        if variant == "sbuf":
            m["wsb"] = tabs[q]
        else:
            m["wq"] = tabs[q]
        in_maps.append(m)
        orders.append(order)

    res = run_bass_kernel_spmd(nc, in_maps, list(range(N_CORES)))

    full = np.empty((BATCH, D), dtype=np.float32)
    for h in range(2):
        acc = np.zeros((HALF, D), dtype=np.float32)
        for q in range(4):
            c = h * 4 + q
            part = res.results[c]["out"]
            acc[orders[c]] += (part if variant in ("q512", "q256")
                               else part.T)
        full[h * HALF:(h + 1) * HALF] = acc
    return full


# revision 21
# speedup vs baseline: 16.5903x; 1.0044x over previous
"""Trainium2 Bass kernel for CatFeaturesItemNet (EmbeddingBag sum, segment_reduce).

Strategy (table-quarter x batch-half grid over 8 cores):
  * Core c handles batch half h = c//4 (32768 items) and vocab quarter
    q = c%4 (25000 rows, padded to 25088 with a zero row used for lane
    padding -> no masks needed anywhere).
  * Host prep per core: gather each item's feature ids, keep the ones in
    quarter q (k_i of them), left-pack, sort items by k_i descending, pad
    each group of 128 items to a static per-group lane budget L_SCHED[g]
    (binomial order-statistic bound; program shape is input-independent).
  * Device: the quarter table is stored as bf16 (either SBUF-resident or
    read directly from HBM); per super-group one dma_gather with
    transpose=True pulls 256B rows => dest[d, slot] = W[token[slot], d]
    with slots item-major, so a contiguous innermost tensor_reduce gives
    the per-item bag sums [d, 128 items] in f32. 4 SWDGE queues round-robin
    (measured ~2-3 ns/descriptor vs ~8 single-queue).
  * Output [128 d, 32768 sorted items] f32 per core; host adds the four
    quarter partials per half and unsorts.
"""

import numpy as np
import ml_dtypes
from contextlib import ExitStack
from math import comb

N_CORES = 8
BATCH = 65536
HALF = BATCH // 2              # items per core (two cores share each half)
L_MAX = 16
D = 128
V = 100000
QROWS = V // 4                 # 25000 rows per vocab quarter
QPAD = 25088                   # padded to rank multiple of 128 (196 ranks)
ZTOK = QROWS                   # first padded row: all-zero, used for padding
GROUPS = HALF // 128           # 256 groups of 128 items
SAFETY_SIGMA = 10.0
MAX_SUPER_SLOTS = 4096         # per-gather-call slot cap (num_idxs)
MAX_SUPER_GROUPS = 8           # per-call group cap (bounds the psum tile)
import os as _os
NQUEUES = int(_os.environ.get("KNQ", "4"))


def _lane_schedule():
    """L_hat[g]: static bound on max per-quarter lane count within group g of
    128 items after sorting counts descending. count k_i ~ Binomial(len_i, 1/4)
    with len_i ~ Uniform{1..16}; count(k >= K) ~ Binomial(HALF, p_K)."""
    def p_ge(K):
        tot = 0.0
        for ln in range(1, L_MAX + 1):
            tot += sum(comb(ln, j) * 0.25 ** j * 0.75 ** (ln - j)
                       for j in range(K, ln + 1)) / 16.0
        return tot

    sched = []
    for g in range(GROUPS):
        lhat = L_MAX
        for K in range(0, L_MAX + 1):
            p = p_ge(K + 1)
            mean = HALF * p
            sig = np.sqrt(HALF * p * (1 - p))
            if mean + SAFETY_SIGMA * sig <= g * 128:
                lhat = K
                break
        sched.append(lhat)
    return sched


L_SCHED = _lane_schedule()


def _super_plan(uniform=False):
    """Pack consecutive non-empty groups into gather calls obeying the slot
    and group caps. uniform=True additionally requires equal L within a call
    (enables contiguous tree-reduction). Returns list of (g0, g1, nidx)."""
    plan = []
    g = 0
    while g < GROUPS and L_SCHED[g] > 0:
        g0 = g
        slots = 0
        while (g < GROUPS and L_SCHED[g] > 0 and g - g0 < MAX_SUPER_GROUPS
               and slots + 128 * L_SCHED[g] <= MAX_SUPER_SLOTS
               and (not uniform or L_SCHED[g] == L_SCHED[g0])):
            slots += 128 * L_SCHED[g]
            g += 1
        plan.append((g0, g, slots))
    return plan


SUPERS = _super_plan()
SLOTS = sum(s for _, _, s in SUPERS)
ICOLS = SLOTS // 16
USUPERS = _super_plan(uniform=True)
USLOTS = sum(s for _, _, s in USUPERS)
UICOLS = USLOTS // 16


def _build_bass(variant="hbm", replicas=1):
    import concourse.bass as bass
    import concourse.bacc as bacc
    import concourse.tile as tile
    from concourse import mybir
    from concourse.library_config import mlp

    nc = bacc.Bacc("TRN2", target_bir_lowering=False, debug=False,
                   num_devices=N_CORES, num_swdge_queues=NQUEUES)
    if variant == "sbuf":
        wsb = nc.declare_dram_parameter("wsb", [128, QPAD],
                                        mybir.dt.bfloat16, isOutput=False)
    elif variant == "hbm":
        wq = nc.declare_dram_parameter("wq", [QPAD, D], mybir.dt.bfloat16,
                                       isOutput=False)
    else:  # q512/q256: f32/bf16 rows, non-transpose gather, tree reduce
        gdt = mybir.dt.float32 if variant == "q512" else mybir.dt.bfloat16
        wq = nc.declare_dram_parameter("wq", [QPAD, D], gdt, isOutput=False)
    tree = variant in ("q512", "q256")
    supers = USUPERS if tree else SUPERS
    icols = UICOLS if tree else ICOLS
    idx_in = nc.declare_dram_parameter("idx", [128, icols], mybir.dt.int16,
                                       isOutput=False)
    if tree:
        out = nc.declare_dram_parameter("out", [HALF, D], mybir.dt.float32,
                                        isOutput=True)
        out_v = out.rearrange("(G p) d -> p G d", p=128)   # [128, GROUPS, D]
    else:
        out = nc.declare_dram_parameter("out", [128, HALF], mybir.dt.float32,
                                        isOutput=True)

    with tile.TileContext(nc) as tc:
        with ExitStack() as ctx:
            cons = ctx.enter_context(tc.tile_pool(name="cons", bufs=1))
            gp = ctx.enter_context(
                tc.tile_pool(name="g", bufs=10 if variant == "q256" else 6))
            op = ctx.enter_context(tc.tile_pool(name="o", bufs=4))

            nc.gpsimd.load_library(mlp)
            idx_t = cons.tile([128, icols], mybir.dt.int16)
            nc.sync.dma_start(out=idx_t[:], in_=idx_in[:, :])

            def body():
                if variant == "sbuf":
                    wtab = cons.tile([128, QPAD], mybir.dt.bfloat16,
                                     tag="wtab")
                    nc.sync.dma_start(out=wtab[:], in_=wsb[:, :])
                ic = 0
                for si, (g0, g1, nidx) in enumerate(supers):
                    qn = si % NQUEUES
                    if tree:
                        L = L_SCHED[g0]
                        ng = g1 - g0
                        gt = gp.tile([128, MAX_SUPER_SLOTS], gdt, tag="g")
                        gv = gt[:, :nidx]
                        nc.gpsimd.dma_gather(
                            out_ap=gv.rearrange("p (c e) -> p c e", e=D),
                            in_ap=wq[:, :],
                            idxs_ap=idx_t[:, ic:ic + nidx // 16],
                            num_idxs=nidx, num_idxs_reg=nidx,
                            elem_size=D, single_packet=False, queue_num=qn)
                        # contiguous tree-reduce over the lane axis t
                        v4 = gv.rearrange("p (g t e) -> p g t e", t=L, e=D)
                        cur = L
                        while cur > 1:
                            lo = (cur + 1) // 2
                            h = cur - lo
                            nc.vector.tensor_tensor(
                                out=v4[:, :, 0:h, :], in0=v4[:, :, 0:h, :],
                                in1=v4[:, :, lo:lo + h, :],
                                op=mybir.AluOpType.add)
                            cur = lo
                        ot = op.tile([128, 128 * MAX_SUPER_GROUPS],
                                     mybir.dt.float32, tag="o")
                        nc.vector.tensor_copy(out=ot[:, :ng * D].rearrange(
                            "p (g e) -> p g e", e=D), in_=v4[:, :, 0, :])
                        nc.sync.dma_start(out=out_v[:, g0:g1, :],
                                          in_=ot[:, :ng * D].rearrange(
                                              "p (g e) -> p g e", e=D))
                        ic += nidx // 16
                        continue
                    gt = gp.tile([128, MAX_SUPER_SLOTS], mybir.dt.bfloat16,
                                 tag="g")
                    gv = gt[:, :nidx]
                    kw = dict(
                        out_ap=gv.rearrange("p (c n) -> p c n", c=1),
                        idxs_ap=idx_t[:, ic:ic + nidx // 16],
                        num_idxs=nidx, num_idxs_reg=nidx,
                        elem_size=128, transpose=True, single_packet=False,
                        queue_num=qn)
                    if variant == "sbuf":
                        nc.gpsimd.dma_gather(
                            in_ap=wtab[:, :], sbuf_tokens_per_rank=128,
                            sbuf_free_dim_per_rank=256, **kw)
                    else:
                        nc.gpsimd.dma_gather(in_ap=wq[:, :], **kw)
                    ot = op.tile([128, 128 * MAX_SUPER_GROUPS],
                                 mybir.dt.float32, tag="o")
                    off = 0
                    for g in range(g0, g1):
                        L = L_SCHED[g]
                        view = gv[:, off:off + 128 * L].rearrange(
                            "d (p t) -> d p t", t=L)
                        nc.vector.tensor_reduce(
                            out=ot[:, (g - g0) * 128:(g - g0 + 1) * 128],
                            in_=view, axis=mybir.AxisListType.X,
                            op=mybir.AluOpType.add)
                        off += 128 * L
                    nc.sync.dma_start(
                        out=out[:, g0 * 128:g1 * 128],
                        in_=ot[:, :(g1 - g0) * 128])
                    ic += nidx // 16

            if replicas == 1:
                body()
            else:
                with tc.For_i(0, replicas):
                    body()
    nc.compile()
    return nc


def _host_prep(items_half, emb_bag_inputs, offsets, input_lengths, quarter,
               variant="hbm"):
    """Per-core index prep. Returns (idx_arr [128, icols] int16, order)."""
    it = items_half.astype(np.int64)
    off = offsets[it].astype(np.int64)
    ln = input_lengths[it].astype(np.int64)
    ids = emb_bag_inputs[off[:, None] + np.arange(L_MAX)[None, :]].astype(
        np.int64)                                          # [N, 16]
    valid = np.arange(L_MAX)[None, :] < ln[:, None]
    inq = valid & (ids // QROWS == quarter)
    local = np.where(inq, ids - QROWS * quarter, 30000)    # pad sorts last
    packed = np.sort(local, axis=1)                        # [N, 16]
    k = inq.sum(axis=1)

    order = np.argsort(-k, kind="stable")
    packed_s = packed[order]
    k_s = k[order]

    tree = variant in ("q512", "q256")
    supers = USUPERS if tree else SUPERS
    icols = UICOLS if tree else ICOLS
    idx_arr = np.zeros((128, icols), dtype=np.int16)
    ic = 0
    for g0, g1, nidx in supers:
        for g in range(g0, g1):
            if k_s[g * 128] > L_SCHED[g]:
                raise RuntimeError(
                    f"lane schedule violated: group {g} max k {k_s[g*128]} "
                    f"> {L_SCHED[g]}")
        if tree:
            L = L_SCHED[g0]
            ng = g1 - g0
            slabs = packed_s[g0 * 128:g1 * 128, :L]
            slabs = np.where(slabs >= QROWS, ZTOK, slabs)
            # stream s = c*128 + p, c = gi*L + t (column-major slots)
            stream = slabs.reshape(ng, 128, L).transpose(0, 2, 1).reshape(-1)
        else:
            streams = []
            for g in range(g0, g1):
                L = L_SCHED[g]
                slab = packed_s[g * 128:(g + 1) * 128, :L]
                slab = np.where(slab >= QROWS, ZTOK, slab)
                streams.append(slab.reshape(-1))           # item-major
            stream = np.concatenate(streams)
        wrapped = stream.reshape(nidx // 16, 16).T         # [16, nidx/16]
        idx_arr[:, ic:ic + nidx // 16] = np.tile(wrapped, (8, 1))
        ic += nidx // 16
    return idx_arr.astype(np.int16), order


def _prep_tables(weight, variant):
    """Per-quarter tables in the layout the device expects."""
    dt = np.float32 if variant == "q512" else ml_dtypes.bfloat16  # q256: bf16
    w16 = weight.astype(dt)
    tabs = []
    for q in range(4):
        wq = np.zeros((QPAD, D), dtype=dt)
        wq[:QROWS] = w16[q * QROWS:(q + 1) * QROWS]
        if variant == "sbuf":
            # wtab[p, r*128 + d] = wq[r*128 + p, d]
            tabs.append(np.ascontiguousarray(
                wq.reshape(QPAD // 128, 128, D).transpose(1, 0, 2)
                .reshape(128, QPAD)))
        else:
            tabs.append(wq)
    return tabs


_CACHE = {}
VARIANT = "q256"


def kernel(items, emb_bag_inputs, offsets, input_lengths, weight):
    from concourse.bass_utils import run_bass_kernel_spmd

    variant = VARIANT
    key = ("nc", variant)
    if key not in _CACHE:
        _CACHE[key] = _build_bass(variant)
    nc = _CACHE[key]

    items = np.asarray(items)
    emb_bag_inputs = np.asarray(emb_bag_inputs)
    offsets = np.asarray(offsets)
    input_lengths = np.asarray(input_lengths)
    weight = np.asarray(weight, dtype=np.float32)

    tabs = _prep_tables(weight, variant)
    in_maps = []
    orders = []
    for c in range(N_CORES):
        h, q = c // 4, c % 4
        idx_arr, order = _host_prep(items[h * HALF:(h + 1) * HALF],
                                    emb_bag_inputs, offsets, input_lengths, q,
                                    variant)
        m = {"idx": idx_arr}
        if variant == "sbuf":
            m["wsb"] = tabs[q]
        else:
            m["wq"] = tabs[q]
        in_maps.append(m)
        orders.append(order)

    res = run_bass_kernel_spmd(nc, in_maps, list(range(N_CORES)))

    full = np.empty((BATCH, D), dtype=np.float32)
    for h in range(2):
        acc = np.zeros((HALF, D), dtype=np.float32)
        for q in range(4):
            c = h * 4 + q
            part = res.results[c]["out"]
            acc[orders[c]] += (part if variant in ("q512", "q256")
                               else part.T)
        full[h * HALF:(h + 1) * HALF] = acc
    return full
